# revision 38
# baseline (speedup 1.0000x reference)
"""Trainium2 Bass kernel for the Coupling-layer mixture-CDF flow problem.

Computes, for inputs x_change,a,b [B,C,L] and pi,mu,s [B,K,C,L]:
    scale   = sigmoid(a + 2) + (1 - sigmoid(2))
    logpi   = log_softmax(pi, axis=K)
    z       = (x - mu) * exp(-s)
    u       = exp(logsumexp_k(logpi + log_sigmoid(z)))
    y       = logit(u)
    log_pdf = logsumexp_k(logpi + z - s - 2*softplus(z))
    out     = (y + b) * scale
    sldj   += sum_{c,l}(log_pdf - log(u) - log(1-u) + log(scale))

Rewritten in an unnormalized, division-free, tanh-only form (so the whole
K-phase stays inside the single ACT table set `exp_and_others`):
    e = exp(pi), t = exp(-s), th = tanh(z/2)
    E = sum_k e, T = sum_k e*th, Dn = sum_k e*t*(th^2 - 1)
    y = ln(E+T) - ln(E-T)
    contrib = ln(-E*Dn) + ln(scale) - ln((E+T)*(E-T))   (constants cancel)

Engine split per k-slice:
    DMA:  pi/mu/s tile loads (HW DGE); d0' = mu - x via a SWDGE
          accumulate-add of a pre-negated x (the sign rides through the
          odd tanh and swaps U2/V2), freeing one DVE pass per slice
    ACT:  exp(pi), exp(-s), tanh(z/2), square
    DVE:  z = d0*t, p = e*th, et = e*t, dp = (th^2-1)*et, E += e
    PE :  T += p and Dn += dp as fp32 identity-matmul PSUM accumulations
L is processed in 2 chunks so each chunk's post phase (logs, out store,
sldj row-sums via accum_out sidecars) overlaps the next chunk's K loop;
the cross-partition sldj reduction is one tiny selector matmul.

Sharding: batch-parallel across 8 NeuronCores, 2 batches/core; on-chip
layout is [128 partitions = (b,c), 2048 free = L].
"""

import sys
import os
import numpy as np

for _p in ("/opt/trn_rl_repo",):
    if _p not in sys.path and os.path.isdir(_p):
        sys.path.insert(0, _p)

B, K, C, L = 16, 8, 64, 2048
NCORES = 8
BPC = B // NCORES          # batches per core
P = BPC * C                # 128 partitions
MMF = 512                  # matmul free-dim chunk (one PSUM bank, fp32)

SIG2 = 1.0 / (1.0 + np.exp(-2.0))          # sigmoid(2)
SCALE_CONST = 0.5 + (1.0 - SIG2)           # added to 0.5*tanh((a+2)/2)

_CACHE = {}


def build_nc():
    """Build + compile the per-core Bass program (identical on all cores)."""
    if "nc" in _CACHE:
        return _CACHE["nc"]

    from contextlib import ExitStack
    import concourse.bass as bass  # noqa: F401
    import concourse.tile as tile
    from concourse import bacc, mybir
    from concourse.masks import make_identity

    f32 = mybir.dt.float32
    Act = mybir.ActivationFunctionType
    Alu = mybir.AluOpType

    nc = bacc.Bacc(
        "TRN2",
        target_bir_lowering=False,
        debug=False,
        enable_asserts=False,
        num_devices=NCORES,
    )

    x_d = nc.dram_tensor("x_change", [BPC, C, L], f32, kind="ExternalInput").ap()
    a_d = nc.dram_tensor("a", [BPC, C, L], f32, kind="ExternalInput").ap()
    b_d = nc.dram_tensor("b", [BPC, C, L], f32, kind="ExternalInput").ap()
    pi_d = nc.dram_tensor("pi", [BPC, K, C, L], f32, kind="ExternalInput").ap()
    mu_d = nc.dram_tensor("mu", [BPC, K, C, L], f32, kind="ExternalInput").ap()
    s_d = nc.dram_tensor("s", [BPC, K, C, L], f32, kind="ExternalInput").ap()
    sldj_d = nc.dram_tensor("sldj", [BPC], f32, kind="ExternalInput").ap()
    out_d = nc.dram_tensor("out", [BPC, C, L], f32, kind="ExternalOutput").ap()
    sldjo_d = nc.dram_tensor("sldj_out", [BPC], f32, kind="ExternalOutput").ap()

    def flat_bc(ap3d):
        # [BPC, C, L] dram AP -> [(b c), L]; valid (uniform stride) for the
        # contiguous x/a/b/out tensors.
        return ap3d.rearrange("b c l -> (b c) l")

    NLC = 2            # L chunks
    LC = L // NLC      # chunk width

    with tile.TileContext(nc) as tc:
        with ExitStack() as ctx:
            cpool = ctx.enter_context(tc.tile_pool(name="cpool", bufs=1))
            iopool = ctx.enter_context(tc.tile_pool(name="iopool", bufs=3))
            wpool = ctx.enter_context(tc.tile_pool(name="wpool", bufs=3))

            # constants
            ident = cpool.tile([P, P], f32, name="ident")
            make_identity(nc, ident[:])
            sel = cpool.tile([P, BPC], f32, name="sel")
            nc.vector.memset(sel[:], 0.0)
            for j in range(BPC):
                nc.vector.memset(sel[j * C : (j + 1) * C, j : j + 1], 1.0)

            # resident inputs (full L)
            xt = cpool.tile([P, L], f32, name="xt")
            nc.sync.dma_start(xt[:], flat_bc(x_d))
            xn = cpool.tile([P, L], f32, name="xn")
            nc.vector.tensor_scalar_mul(xn[:], xt[:], -1.0)
            at = cpool.tile([P, L], f32, name="at")
            nc.sync.dma_start(at[:], flat_bc(a_d))
            bt = cpool.tile([P, L], f32, name="bt")
            nc.sync.dma_start(bt[:], flat_bc(b_d))

            # SBUF accumulator E (full L); T/Dn accumulate per-chunk in PSUM
            E = cpool.tile([P, L], f32, name="E")
            rs_tot = cpool.tile([P, 1], f32, name="rs_tot")

            def front(lc, k):
                """loads + d0 + exps + z for (chunk lc, mixture k)."""
                csl = slice(lc * LC, (lc + 1) * LC)
                pi_t = iopool.tile([P, LC], f32, name=f"pi{lc}_{k}", tag="pi_t")
                for bb in range(BPC):
                    nc.sync.dma_start(
                        pi_t[bb * C : (bb + 1) * C, :], pi_d[bb, k, :, csl]
                    )
                mu_t = iopool.tile([P, LC], f32, name=f"mu{lc}_{k}", tag="mu_t")
                for bb in range(BPC):
                    nc.sync.dma_start(
                        mu_t[bb * C : (bb + 1) * C, :], mu_d[bb, k, :, csl]
                    )
                s_t = iopool.tile([P, LC], f32, name=f"s{lc}_{k}", tag="s_t")
                for bb in range(BPC):
                    nc.sync.dma_start(
                        s_t[bb * C : (bb + 1) * C, :], s_d[bb, k, :, csl]
                    )
                # d0' = mu - x  (SWDGE accumulate-add of -x onto mu;
                # the sign rides through the odd tanh and swaps U2/V2)
                nc.gpsimd.dma_start(mu_t[:], xn[:, csl], accum_op=Alu.add)
                # e = exp(pi)           (in-place on pi tile)
                nc.scalar.activation(pi_t[:], pi_t[:], Act.Exp)
                # t = exp(-s)           (in-place on s tile)
                nc.scalar.activation(s_t[:], s_t[:], Act.Exp, scale=-1.0)
                # z = d0 * t            (in-place on mu tile)
                nc.vector.tensor_mul(mu_t[:], mu_t[:], s_t[:])
                return pi_t, mu_t, s_t

            def back(lc, k, tiles, T_ps, Dn_ps):
                csl = slice(lc * LC, (lc + 1) * LC)
                pi_t, mu_t, s_t = tiles
                # th = tanh(z/2)        (in-place on mu tile)
                nc.scalar.activation(mu_t[:], mu_t[:], Act.Tanh, scale=0.5)
                # s2 = th^2
                s2_t = wpool.tile([P, LC], f32, name=f"s2{lc}_{k}", tag="s2_t")
                nc.scalar.activation(s2_t[:], mu_t[:], Act.Square)
                # et = e * t   (bf16: feeds only the sldj Dn path)
                et_t = wpool.tile([P, LC], f32, name=f"et{lc}_{k}", tag="et_t")
                nc.vector.tensor_mul(et_t[:], pi_t[:], s_t[:])
                # E += e  (DVE, fp32)
                if k == 0:
                    nc.vector.tensor_copy(E[:, csl], pi_t[:])
                else:
                    nc.vector.tensor_add(E[:, csl], E[:, csl], pi_t[:])
                # p = e * th
                p_t = wpool.tile([P, LC], f32, name=f"p{lc}_{k}", tag="p_t")
                nc.vector.tensor_mul(p_t[:], pi_t[:], mu_t[:])
                # dp = (th^2 - 1) * et
                dp_t = wpool.tile([P, LC], f32, name=f"dp{lc}_{k}", tag="dp_t")
                nc.vector.scalar_tensor_tensor(
                    dp_t[:], s2_t[:], 1.0, et_t[:],
                    op0=Alu.subtract, op1=Alu.mult,
                )
                # T += p (PE fp32), Dn += dp (PE bf16 products, fp32 PSUM)
                for j in range(LC // MMF):
                    sl = slice(j * MMF, (j + 1) * MMF)
                    nc.tensor.matmul(
                        T_ps[:, sl], ident[:], p_t[:, sl],
                        start=(k == 0), stop=(k == K - 1),
                    )
                    nc.tensor.matmul(
                        Dn_ps[:, sl], ident[:], dp_t[:, sl],
                        start=(k == 0), stop=(k == K - 1),
                    )

            def post(lc, T_ps, Dn_ps):
                csl = slice(lc * LC, (lc + 1) * LC)
                a_s, b_s, e_s = at[:, csl], bt[:, csl], E[:, csl]
                # scale = 0.5*tanh((a+2)/2) + (1.5 - sigmoid(2))  (in-place)
                nc.scalar.activation(a_s, a_s, Act.Tanh, scale=0.5, bias=1.0)
                nc.vector.tensor_scalar(
                    a_s, a_s, 0.5, SCALE_CONST, op0=Alu.mult, op1=Alu.add
                )
                U2 = iopool.tile([P, LC], f32, name=f"U2_{lc}", tag="pi_t")
                V2 = iopool.tile([P, LC], f32, name=f"V2_{lc}", tag="mu_t")
                nc.vector.tensor_sub(U2[:], e_s, T_ps[:])   # E+T = 2*U (T'=-T)
                nc.vector.tensor_add(V2[:], e_s, T_ps[:])   # E-T = 2*V
                nc.vector.tensor_mul(e_s, e_s, Dn_ps[:])    # P1 = E*Dn (<0)
                T2 = iopool.tile([P, LC], f32, name=f"T2_{lc}", tag="s_t")
                nc.vector.tensor_mul(T2[:], U2[:], V2[:])   # P2 = (E+T)(E-T)

                nc.scalar.activation(U2[:], U2[:], Act.Ln)           # ln(E+T)
                nc.scalar.activation(V2[:], V2[:], Act.Ln)           # ln(E-T)
                nc.scalar.activation(e_s, e_s, Act.Ln, scale=-1.0)   # ln(-E*Dn)
                nc.scalar.activation(T2[:], T2[:], Act.Ln)           # ln(P2)

                # y = ln(E+T) - ln(E-T)
                yt = wpool.tile([P, LC], f32, name=f"yt_{lc}", tag="p_t")
                nc.vector.tensor_sub(yt[:], U2[:], V2[:])

                rs1 = wpool.tile([P, 1], f32, name=f"rs1_{lc}", tag="rs1")
                rs2 = wpool.tile([P, 1], f32, name=f"rs2_{lc}", tag="rs2")
                # c1 = lnP1 - lnP2, rowsum -> rs1    (into U2 tile)
                nc.vector.scalar_tensor_tensor(
                    U2[:], T2[:], -1.0, e_s,
                    op0=Alu.mult, op1=Alu.add, accum_out=rs1[:],
                )
                # lsc = ln(scale), rowsum -> rs2     (into V2 tile)
                nc.scalar.activation(V2[:], a_s, Act.Ln, accum_out=rs2[:])

                # out = (y + b) * scale              (in-place on bt slice)
                nc.vector.tensor_add(b_s, yt[:], b_s)
                nc.vector.tensor_mul(b_s, b_s, a_s)
                nc.sync.dma_start(flat_bc(out_d)[:, csl], b_s)

                nc.vector.tensor_add(rs1[:], rs1[:], rs2[:])
                if lc == 0:
                    nc.vector.tensor_copy(rs_tot[:], rs1[:])
                else:
                    nc.vector.tensor_add(rs_tot[:], rs_tot[:], rs1[:])

            with tc.tile_pool(name="pspool", bufs=2, space="PSUM") as pspool:
                steps = [(lc, k) for lc in range(NLC) for k in range(K)]
                tiles = front(*steps[0])
                ps = {}
                for si, (lc, k) in enumerate(steps):
                    if k == 0:
                        ps[lc] = (
                            pspool.tile([P, LC], f32, name=f"T_ps{lc}", tag="T_ps"),
                            pspool.tile([P, LC], f32, name=f"D_ps{lc}", tag="D_ps"),
                        )
                    nxt = front(*steps[si + 1]) if si + 1 < len(steps) else None
                    back(lc, k, tiles, *ps[lc])
                    tiles = nxt
                    if k == K - 1:
                        post(lc, *ps[lc])

            # sldj: per-batch sums via tiny selector matmul
            with tc.tile_pool(name="ps2pool", bufs=1, space="PSUM") as ps2pool:
                ps2 = ps2pool.tile([BPC, 1], f32, name="ps2")
                nc.tensor.matmul(ps2[:], sel[:], rs_tot[:], start=True, stop=True)
                res = cpool.tile([BPC, 1], f32, name="res")
                nc.vector.tensor_copy(res[:], ps2[:])
            si_t = cpool.tile([BPC, 1], f32, name="si_t")
            nc.sync.dma_start(si_t[:], sldj_d.rearrange("(b o) -> b o", o=1))
            nc.vector.tensor_add(res[:], res[:], si_t[:])
            nc.sync.dma_start(sldjo_d.rearrange("(b o) -> b o", o=1), res[:])

    nc.compile()
    _CACHE["nc"] = nc
    return nc


def make_in_maps(x_change, a, b, pi, mu, s, sldj):
    x_change = np.ascontiguousarray(x_change, dtype=np.float32)
    a = np.ascontiguousarray(a, dtype=np.float32)
    b = np.ascontiguousarray(b, dtype=np.float32)
    pi = np.ascontiguousarray(pi, dtype=np.float32)
    mu = np.ascontiguousarray(mu, dtype=np.float32)
    s = np.ascontiguousarray(s, dtype=np.float32)
    sldj = np.ascontiguousarray(sldj, dtype=np.float32)
    in_maps = []
    for i in range(NCORES):
        sl = slice(i * BPC, (i + 1) * BPC)
        in_maps.append(
            {
                "x_change": np.ascontiguousarray(x_change[sl]),
                "a": np.ascontiguousarray(a[sl]),
                "b": np.ascontiguousarray(b[sl]),
                "pi": np.ascontiguousarray(pi[sl]),
                "mu": np.ascontiguousarray(mu[sl]),
                "s": np.ascontiguousarray(s[sl]),
                "sldj": np.ascontiguousarray(sldj[sl]),
            }
        )
    return in_maps


def run_shards(in_maps, trace=False, **kwargs):
    from concourse.bass_utils import run_bass_kernel_spmd

    nc = build_nc()
    return run_bass_kernel_spmd(
        nc, in_maps, core_ids=list(range(NCORES)), trace=trace, **kwargs
    )


def assemble(results):
    out = np.concatenate([r["out"] for r in results], axis=0)
    sldj_out = np.concatenate([r["sldj_out"] for r in results], axis=0)
    return out, sldj_out


def _get_runner():
    """jit the SPMD executable once (same lowering as run_bass_kernel_spmd's
    axon path) so repeat kernel() calls skip the per-call retrace."""
    if "runner" in _CACHE:
        return _CACHE["runner"]

    import jax
    from jax.sharding import Mesh, PartitionSpec
    from jax.experimental.shard_map import shard_map
    from concourse import bass2jax as b2j
    from concourse import mybir

    nc = build_nc()
    b2j.install_neuronx_cc_hook()

    partition_name = nc.partition_id_tensor.name if nc.partition_id_tensor else None
    in_names, out_names, out_avals = [], [], []
    for alloc in nc.m.functions[0].allocations:
        if not isinstance(alloc, mybir.MemoryLocationSet):
            continue
        name = alloc.memorylocations[0].name
        if alloc.kind == "ExternalInput":
            if name != partition_name:
                in_names.append(name)
        elif alloc.kind == "ExternalOutput":
            out_names.append(name)
            out_avals.append(
                jax.core.ShapedArray(
                    tuple(alloc.tensor_shape), mybir.dt.np(alloc.dtype)
                )
            )
    n_params = len(in_names)
    all_names = in_names + out_names
    if partition_name is not None:
        all_names = all_names + [partition_name]

    def _body(*args):
        operands = list(args)
        if partition_name is not None:
            operands.append(b2j.partition_id_tensor())
        return tuple(
            b2j._bass_exec_p.bind(
                *operands,
                out_avals=tuple(out_avals),
                in_names=tuple(all_names),
                out_names=tuple(out_names),
                lowering_input_output_aliases=(),
                sim_require_finite=True,
                sim_require_nnan=True,
                nc=nc,
            )
        )

    devices = jax.devices()[:NCORES]
    mesh = Mesh(np.asarray(devices), ("core",))
    n_out = len(out_names)
    fn = jax.jit(
        shard_map(
            _body,
            mesh=mesh,
            in_specs=(PartitionSpec("core"),) * (n_params + n_out),
            out_specs=(PartitionSpec("core"),) * n_out,
            check_rep=False,
        ),
        keep_unused=True,
    )
    runner = (fn, in_names, out_names, out_avals)
    _CACHE["runner"] = runner
    return runner


def kernel(x_change, a, b, pi, mu, s, sldj):
    in_maps = make_in_maps(x_change, a, b, pi, mu, s, sldj)
    try:
        fn, in_names, out_names, out_avals = _get_runner()
        concat_in = [
            np.concatenate([m[nm] for m in in_maps], axis=0) for nm in in_names
        ]
        concat_zero = [
            np.zeros((NCORES * av.shape[0], *av.shape[1:]), av.dtype)
            for av in out_avals
        ]
        outs = fn(*concat_in, *concat_zero)
        by_name = {nm: np.asarray(o) for nm, o in zip(out_names, outs)}
        return by_name["out"], by_name["sldj_out"]
    except Exception:
        res = run_shards(in_maps, trace=False)
        return assemble(res.results)


# revision 41
# speedup vs baseline: 1.0020x; 1.0020x over previous
"""Trainium2 Bass kernel for the Coupling-layer mixture-CDF flow problem.

Computes, for inputs x_change,a,b [B,C,L] and pi,mu,s [B,K,C,L]:
    scale   = sigmoid(a + 2) + (1 - sigmoid(2))
    logpi   = log_softmax(pi, axis=K)
    z       = (x - mu) * exp(-s)
    u       = exp(logsumexp_k(logpi + log_sigmoid(z)))
    y       = logit(u)
    log_pdf = logsumexp_k(logpi + z - s - 2*softplus(z))
    out     = (y + b) * scale
    sldj   += sum_{c,l}(log_pdf - log(u) - log(1-u) + log(scale))

Rewritten in an unnormalized, division-free, tanh-only form (so the whole
K-phase stays inside the single ACT table set `exp_and_others`):
    e = exp(pi), t = exp(-s), th = tanh(z/2)
    E = sum_k e, T = sum_k e*th, Dn = sum_k e*t*(th^2 - 1)
    y = ln(E+T) - ln(E-T)
    contrib = ln(-E*Dn) + ln(scale) - ln((E+T)*(E-T))   (constants cancel)

Engine split per k-slice:
    DMA:  pi/mu/s tile loads (HW DGE); d0' = mu - x via a SWDGE
          accumulate-add of a pre-negated x (the sign rides through the
          odd tanh and swaps U2/V2), freeing one DVE pass per slice
    ACT:  exp(pi), exp(-s), tanh(z/2), square
    DVE:  z = d0*t, p = e*th, et = e*t, dp = (th^2-1)*et, E += e
    PE :  T += p and Dn += dp as fp32 identity-matmul PSUM accumulations
          (T matmuls grouped before Dn per slice: PE starts on p without
          waiting for the later dp)
L is processed in 2 chunks so each chunk's post phase (logs, out store,
sldj row-sums via accum_out sidecars) overlaps the next chunk's K loop;
the cross-partition sldj reduction is one tiny selector matmul.

Sharding: batch-parallel across 8 NeuronCores, 2 batches/core; on-chip
layout is [128 partitions = (b,c), 2048 free = L].
"""

import sys
import os
import numpy as np

for _p in ("/opt/trn_rl_repo",):
    if _p not in sys.path and os.path.isdir(_p):
        sys.path.insert(0, _p)

B, K, C, L = 16, 8, 64, 2048
NCORES = 8
BPC = B // NCORES          # batches per core
P = BPC * C                # 128 partitions
MMF = 512                  # matmul free-dim chunk (one PSUM bank, fp32)

SIG2 = 1.0 / (1.0 + np.exp(-2.0))          # sigmoid(2)
SCALE_CONST = 0.5 + (1.0 - SIG2)           # added to 0.5*tanh((a+2)/2)

_CACHE = {}


def build_nc():
    """Build + compile the per-core Bass program (identical on all cores)."""
    if "nc" in _CACHE:
        return _CACHE["nc"]

    from contextlib import ExitStack
    import concourse.bass as bass  # noqa: F401
    import concourse.tile as tile
    from concourse import bacc, mybir
    from concourse.masks import make_identity

    f32 = mybir.dt.float32
    Act = mybir.ActivationFunctionType
    Alu = mybir.AluOpType

    nc = bacc.Bacc(
        "TRN2",
        target_bir_lowering=False,
        debug=False,
        enable_asserts=False,
        num_devices=NCORES,
    )

    x_d = nc.dram_tensor("x_change", [BPC, C, L], f32, kind="ExternalInput").ap()
    a_d = nc.dram_tensor("a", [BPC, C, L], f32, kind="ExternalInput").ap()
    b_d = nc.dram_tensor("b", [BPC, C, L], f32, kind="ExternalInput").ap()
    pi_d = nc.dram_tensor("pi", [BPC, K, C, L], f32, kind="ExternalInput").ap()
    mu_d = nc.dram_tensor("mu", [BPC, K, C, L], f32, kind="ExternalInput").ap()
    s_d = nc.dram_tensor("s", [BPC, K, C, L], f32, kind="ExternalInput").ap()
    sldj_d = nc.dram_tensor("sldj", [BPC], f32, kind="ExternalInput").ap()
    out_d = nc.dram_tensor("out", [BPC, C, L], f32, kind="ExternalOutput").ap()
    sldjo_d = nc.dram_tensor("sldj_out", [BPC], f32, kind="ExternalOutput").ap()

    def flat_bc(ap3d):
        # [BPC, C, L] dram AP -> [(b c), L]; valid (uniform stride) for the
        # contiguous x/a/b/out tensors.
        return ap3d.rearrange("b c l -> (b c) l")

    NLC = 2            # L chunks
    LC = L // NLC      # chunk width

    with tile.TileContext(nc) as tc:
        with ExitStack() as ctx:
            cpool = ctx.enter_context(tc.tile_pool(name="cpool", bufs=1))
            iopool = ctx.enter_context(tc.tile_pool(name="iopool", bufs=3))
            wpool = ctx.enter_context(tc.tile_pool(name="wpool", bufs=3))

            # constants
            ident = cpool.tile([P, P], f32, name="ident")
            make_identity(nc, ident[:])
            sel = cpool.tile([P, BPC], f32, name="sel")
            nc.vector.memset(sel[:], 0.0)
            for j in range(BPC):
                nc.vector.memset(sel[j * C : (j + 1) * C, j : j + 1], 1.0)

            # resident inputs (full L)
            xt = cpool.tile([P, L], f32, name="xt")
            nc.sync.dma_start(xt[:], flat_bc(x_d))
            xn = cpool.tile([P, L], f32, name="xn")
            nc.vector.tensor_scalar_mul(xn[:], xt[:], -1.0)
            at = cpool.tile([P, L], f32, name="at")
            nc.sync.dma_start(at[:], flat_bc(a_d))
            bt = cpool.tile([P, L], f32, name="bt")
            nc.sync.dma_start(bt[:], flat_bc(b_d))

            # SBUF accumulator E (full L); T/Dn accumulate per-chunk in PSUM
            E = cpool.tile([P, L], f32, name="E")
            rs_tot = cpool.tile([P, 1], f32, name="rs_tot")

            def front(lc, k):
                """loads + d0 + exps + z for (chunk lc, mixture k)."""
                csl = slice(lc * LC, (lc + 1) * LC)
                pi_t = iopool.tile([P, LC], f32, name=f"pi{lc}_{k}", tag="pi_t")
                for bb in range(BPC):
                    nc.sync.dma_start(
                        pi_t[bb * C : (bb + 1) * C, :], pi_d[bb, k, :, csl]
                    )
                mu_t = iopool.tile([P, LC], f32, name=f"mu{lc}_{k}", tag="mu_t")
                for bb in range(BPC):
                    nc.sync.dma_start(
                        mu_t[bb * C : (bb + 1) * C, :], mu_d[bb, k, :, csl]
                    )
                s_t = iopool.tile([P, LC], f32, name=f"s{lc}_{k}", tag="s_t")
                for bb in range(BPC):
                    nc.sync.dma_start(
                        s_t[bb * C : (bb + 1) * C, :], s_d[bb, k, :, csl]
                    )
                # d0' = mu - x  (SWDGE accumulate-add of -x onto mu;
                # the sign rides through the odd tanh and swaps U2/V2)
                nc.gpsimd.dma_start(mu_t[:], xn[:, csl], accum_op=Alu.add)
                # e = exp(pi)           (in-place on pi tile)
                nc.scalar.activation(pi_t[:], pi_t[:], Act.Exp)
                # t = exp(-s)           (in-place on s tile)
                nc.scalar.activation(s_t[:], s_t[:], Act.Exp, scale=-1.0)
                # z = d0 * t            (in-place on mu tile)
                nc.vector.tensor_mul(mu_t[:], mu_t[:], s_t[:])
                return pi_t, mu_t, s_t

            def back(lc, k, tiles, T_ps, Dn_ps):
                csl = slice(lc * LC, (lc + 1) * LC)
                pi_t, mu_t, s_t = tiles
                # th = tanh(z/2)        (in-place on mu tile)
                nc.scalar.activation(mu_t[:], mu_t[:], Act.Tanh, scale=0.5)
                # s2 = th^2
                s2_t = wpool.tile([P, LC], f32, name=f"s2{lc}_{k}", tag="s2_t")
                nc.scalar.activation(s2_t[:], mu_t[:], Act.Square)
                # et = e * t   (bf16: feeds only the sldj Dn path)
                et_t = wpool.tile([P, LC], f32, name=f"et{lc}_{k}", tag="et_t")
                nc.vector.tensor_mul(et_t[:], pi_t[:], s_t[:])
                # E += e  (DVE, fp32)
                if k == 0:
                    nc.vector.tensor_copy(E[:, csl], pi_t[:])
                else:
                    nc.vector.tensor_add(E[:, csl], E[:, csl], pi_t[:])
                # p = e * th
                p_t = wpool.tile([P, LC], f32, name=f"p{lc}_{k}", tag="p_t")
                nc.vector.tensor_mul(p_t[:], pi_t[:], mu_t[:])
                # dp = (th^2 - 1) * et
                dp_t = wpool.tile([P, LC], f32, name=f"dp{lc}_{k}", tag="dp_t")
                nc.vector.scalar_tensor_tensor(
                    dp_t[:], s2_t[:], 1.0, et_t[:],
                    op0=Alu.subtract, op1=Alu.mult,
                )
                # T += p (PE fp32), Dn += dp (PE bf16 products, fp32 PSUM)
                for j in range(LC // MMF):
                    sl = slice(j * MMF, (j + 1) * MMF)
                    nc.tensor.matmul(
                        T_ps[:, sl], ident[:], p_t[:, sl],
                        start=(k == 0), stop=(k == K - 1),
                    )
                for j in range(LC // MMF):
                    sl = slice(j * MMF, (j + 1) * MMF)
                    nc.tensor.matmul(
                        Dn_ps[:, sl], ident[:], dp_t[:, sl],
                        start=(k == 0), stop=(k == K - 1),
                    )

            def post(lc, T_ps, Dn_ps):
                csl = slice(lc * LC, (lc + 1) * LC)
                a_s, b_s, e_s = at[:, csl], bt[:, csl], E[:, csl]
                # scale = 0.5*tanh((a+2)/2) + (1.5 - sigmoid(2))  (in-place)
                nc.scalar.activation(a_s, a_s, Act.Tanh, scale=0.5, bias=1.0)
                nc.vector.tensor_scalar(
                    a_s, a_s, 0.5, SCALE_CONST, op0=Alu.mult, op1=Alu.add
                )
                U2 = iopool.tile([P, LC], f32, name=f"U2_{lc}", tag="pi_t")
                V2 = iopool.tile([P, LC], f32, name=f"V2_{lc}", tag="mu_t")
                nc.vector.tensor_sub(U2[:], e_s, T_ps[:])   # E+T = 2*U (T'=-T)
                nc.vector.tensor_add(V2[:], e_s, T_ps[:])   # E-T = 2*V
                nc.vector.tensor_mul(e_s, e_s, Dn_ps[:])    # P1 = E*Dn (<0)
                T2 = iopool.tile([P, LC], f32, name=f"T2_{lc}", tag="s_t")
                nc.vector.tensor_mul(T2[:], U2[:], V2[:])   # P2 = (E+T)(E-T)

                nc.scalar.activation(U2[:], U2[:], Act.Ln)           # ln(E+T)
                nc.scalar.activation(V2[:], V2[:], Act.Ln)           # ln(E-T)
                nc.scalar.activation(e_s, e_s, Act.Ln, scale=-1.0)   # ln(-E*Dn)
                nc.scalar.activation(T2[:], T2[:], Act.Ln)           # ln(P2)

                # y = ln(E+T) - ln(E-T)
                yt = wpool.tile([P, LC], f32, name=f"yt_{lc}", tag="p_t")
                nc.vector.tensor_sub(yt[:], U2[:], V2[:])

                rs1 = wpool.tile([P, 1], f32, name=f"rs1_{lc}", tag="rs1")
                rs2 = wpool.tile([P, 1], f32, name=f"rs2_{lc}", tag="rs2")
                # c1 = lnP1 - lnP2, rowsum -> rs1    (into U2 tile)
                nc.vector.scalar_tensor_tensor(
                    U2[:], T2[:], -1.0, e_s,
                    op0=Alu.mult, op1=Alu.add, accum_out=rs1[:],
                )
                # lsc = ln(scale), rowsum -> rs2     (into V2 tile)
                nc.scalar.activation(V2[:], a_s, Act.Ln, accum_out=rs2[:])

                # out = (y + b) * scale              (in-place on bt slice)
                nc.vector.tensor_add(b_s, yt[:], b_s)
                nc.vector.tensor_mul(b_s, b_s, a_s)
                nc.sync.dma_start(flat_bc(out_d)[:, csl], b_s)

                nc.vector.tensor_add(rs1[:], rs1[:], rs2[:])
                if lc == 0:
                    nc.vector.tensor_copy(rs_tot[:], rs1[:])
                else:
                    nc.vector.tensor_add(rs_tot[:], rs_tot[:], rs1[:])

            with tc.tile_pool(name="pspool", bufs=2, space="PSUM") as pspool:
                steps = [(lc, k) for lc in range(NLC) for k in range(K)]
                tiles = front(*steps[0])
                ps = {}
                for si, (lc, k) in enumerate(steps):
                    if k == 0:
                        ps[lc] = (
                            pspool.tile([P, LC], f32, name=f"T_ps{lc}", tag="T_ps"),
                            pspool.tile([P, LC], f32, name=f"D_ps{lc}", tag="D_ps"),
                        )
                    nxt = front(*steps[si + 1]) if si + 1 < len(steps) else None
                    back(lc, k, tiles, *ps[lc])
                    tiles = nxt
                    if k == K - 1:
                        post(lc, *ps[lc])

            # sldj: per-batch sums via tiny selector matmul
            with tc.tile_pool(name="ps2pool", bufs=1, space="PSUM") as ps2pool:
                ps2 = ps2pool.tile([BPC, 1], f32, name="ps2")
                nc.tensor.matmul(ps2[:], sel[:], rs_tot[:], start=True, stop=True)
                res = cpool.tile([BPC, 1], f32, name="res")
                nc.vector.tensor_copy(res[:], ps2[:])
            si_t = cpool.tile([BPC, 1], f32, name="si_t")
            nc.sync.dma_start(si_t[:], sldj_d.rearrange("(b o) -> b o", o=1))
            nc.vector.tensor_add(res[:], res[:], si_t[:])
            nc.sync.dma_start(sldjo_d.rearrange("(b o) -> b o", o=1), res[:])

    nc.compile()
    _CACHE["nc"] = nc
    return nc


def make_in_maps(x_change, a, b, pi, mu, s, sldj):
    x_change = np.ascontiguousarray(x_change, dtype=np.float32)
    a = np.ascontiguousarray(a, dtype=np.float32)
    b = np.ascontiguousarray(b, dtype=np.float32)
    pi = np.ascontiguousarray(pi, dtype=np.float32)
    mu = np.ascontiguousarray(mu, dtype=np.float32)
    s = np.ascontiguousarray(s, dtype=np.float32)
    sldj = np.ascontiguousarray(sldj, dtype=np.float32)
    in_maps = []
    for i in range(NCORES):
        sl = slice(i * BPC, (i + 1) * BPC)
        in_maps.append(
            {
                "x_change": np.ascontiguousarray(x_change[sl]),
                "a": np.ascontiguousarray(a[sl]),
                "b": np.ascontiguousarray(b[sl]),
                "pi": np.ascontiguousarray(pi[sl]),
                "mu": np.ascontiguousarray(mu[sl]),
                "s": np.ascontiguousarray(s[sl]),
                "sldj": np.ascontiguousarray(sldj[sl]),
            }
        )
    return in_maps


def run_shards(in_maps, trace=False, **kwargs):
    from concourse.bass_utils import run_bass_kernel_spmd

    nc = build_nc()
    return run_bass_kernel_spmd(
        nc, in_maps, core_ids=list(range(NCORES)), trace=trace, **kwargs
    )


def assemble(results):
    out = np.concatenate([r["out"] for r in results], axis=0)
    sldj_out = np.concatenate([r["sldj_out"] for r in results], axis=0)
    return out, sldj_out


def _get_runner():
    """jit the SPMD executable once (same lowering as run_bass_kernel_spmd's
    axon path) so repeat kernel() calls skip the per-call retrace."""
    if "runner" in _CACHE:
        return _CACHE["runner"]

    import jax
    from jax.sharding import Mesh, PartitionSpec
    from jax.experimental.shard_map import shard_map
    from concourse import bass2jax as b2j
    from concourse import mybir

    nc = build_nc()
    b2j.install_neuronx_cc_hook()

    partition_name = nc.partition_id_tensor.name if nc.partition_id_tensor else None
    in_names, out_names, out_avals = [], [], []
    for alloc in nc.m.functions[0].allocations:
        if not isinstance(alloc, mybir.MemoryLocationSet):
            continue
        name = alloc.memorylocations[0].name
        if alloc.kind == "ExternalInput":
            if name != partition_name:
                in_names.append(name)
        elif alloc.kind == "ExternalOutput":
            out_names.append(name)
            out_avals.append(
                jax.core.ShapedArray(
                    tuple(alloc.tensor_shape), mybir.dt.np(alloc.dtype)
                )
            )
    n_params = len(in_names)
    all_names = in_names + out_names
    if partition_name is not None:
        all_names = all_names + [partition_name]

    def _body(*args):
        operands = list(args)
        if partition_name is not None:
            operands.append(b2j.partition_id_tensor())
        return tuple(
            b2j._bass_exec_p.bind(
                *operands,
                out_avals=tuple(out_avals),
                in_names=tuple(all_names),
                out_names=tuple(out_names),
                lowering_input_output_aliases=(),
                sim_require_finite=True,
                sim_require_nnan=True,
                nc=nc,
            )
        )

    devices = jax.devices()[:NCORES]
    mesh = Mesh(np.asarray(devices), ("core",))
    n_out = len(out_names)
    fn = jax.jit(
        shard_map(
            _body,
            mesh=mesh,
            in_specs=(PartitionSpec("core"),) * (n_params + n_out),
            out_specs=(PartitionSpec("core"),) * n_out,
            check_rep=False,
        ),
        keep_unused=True,
    )
    runner = (fn, in_names, out_names, out_avals)
    _CACHE["runner"] = runner
    return runner


def kernel(x_change, a, b, pi, mu, s, sldj):
    in_maps = make_in_maps(x_change, a, b, pi, mu, s, sldj)
    try:
        fn, in_names, out_names, out_avals = _get_runner()
        concat_in = [
            np.concatenate([m[nm] for m in in_maps], axis=0) for nm in in_names
        ]
        concat_zero = [
            np.zeros((NCORES * av.shape[0], *av.shape[1:]), av.dtype)
            for av in out_avals
        ]
        outs = fn(*concat_in, *concat_zero)
        by_name = {nm: np.asarray(o) for nm, o in zip(out_names, outs)}
        return by_name["out"], by_name["sldj_out"]
    except Exception:
        res = run_shards(in_maps, trace=False)
        return assemble(res.results)
